# revision 22
# baseline (speedup 1.0000x reference)
"""Trainium2 Bass kernel for the grouped linear ensemble (moe_routing).

Problem: x [262144, 256] f32, Ws [64, 4, 256, 256], bs [64, 4, 256].
Model m applies its 4-layer stack (h = h @ W_l + b_l) to its contiguous
4096-row slice of x.

Sharding: expert parallel — core c owns models 8c..8c+7 and their rows.
No cross-device communication.

Per-core kernel design:
- Activations are kept feature-on-partitions ("transposed", hT [256, t]) so
  the contraction dim of every matmul lands on the partition axis; the
  orientation is self-consistent across chained layers (h'T = W.T @ hT).
- Input x is transposed on-chip via PE transpose-mode matmuls (exact in
  fp32), 128x128 blocks, PSUM -> SBUF copy casts to float32r.
- Layers 1-3: stationary = weight block (reused across the whole chunk),
  moving = hT, N=512 columns per matmul, fp32r (1 cycle/row at N>=512,
  ~1.5e-4 relative rounding).  Bias is fused into the PSUM->SBUF copy on
  the scalar engine (per-partition bias = per-output-feature).
- Layer 4 uses the activations as the stationary operand and streams W4,
  which makes the output land in natural row-major orientation — no
  output transpose.  Its bias is added with a K=1 ones-outer-product
  matmul accumulated into the same PSUM group.
"""

from contextlib import ExitStack

import numpy as np

import concourse.tile as tile
import concourse.mybir as mybir
from concourse import bacc
from concourse.bass_utils import run_bass_kernel_spmd
from concourse.masks import make_identity

N_CORES = 8
N_MODELS = 64
N_LAYERS = 4
F = 256
ROWS_PER_MODEL = 4096
M_PER_CORE = N_MODELS // N_CORES          # 8 models per core
ROWS_PER_CORE = M_PER_CORE * ROWS_PER_MODEL  # 32768
CHUNK = 512                               # rows of x processed per pipeline step
CHUNKS_PER_MODEL = ROWS_PER_MODEL // CHUNK   # 8

F32 = mybir.dt.float32
F32R = mybir.dt.float32r
COPY = mybir.ActivationFunctionType.Copy
IDENT = mybir.ActivationFunctionType.Identity


def emit_core_kernel(tc, x_d, wc_d, bcr_d, y_d, reps=1):
    nc = tc.nc

    ctx = ExitStack()
    const = ctx.enter_context(tc.tile_pool(name="const", bufs=1))
    wpool = ctx.enter_context(tc.tile_pool(name="w", bufs=2))
    xpool = ctx.enter_context(tc.tile_pool(name="xio", bufs=3))
    hpool = ctx.enter_context(tc.tile_pool(name="h", bufs=3))
    psT = ctx.enter_context(tc.tile_pool(name="psT", bufs=2, space="PSUM"))
    psL4 = ctx.enter_context(tc.tile_pool(name="psL4", bufs=4, space="PSUM"))

    ident = const.tile([128, 128], F32)
    make_identity(nc, ident[:])
    ones_f = const.tile([1, 128], F32)
    nc.gpsimd.memset(ones_f[:], 1.0)
    onesr = const.tile([1, 128], F32R)
    nc.vector.tensor_copy(onesr[:], ones_f[:])

    def body():
      for m in range(M_PER_CORE):
        # --- per-model composed weights (double-buffered across models) ---
        # Wc = W1@W2@W3@W4 composed in float64 on the host and rounded to
        # fp32r, so loads are pure HWDGE copies.  wc[fb] = [128 (f), 256 (g)].
        wc = []
        for fb in range(2):
            wr = wpool.tile([128, F], F32R, tag=f"wr_{fb}")
            nc.sync.dma_start(wr[:], wc_d[m, fb * 128:(fb + 1) * 128, :])
            wc.append(wr)
        # composed bias (host-rounded fp32r): [1, 256] rhs of the K=1 bias MM
        bcr = wpool.tile([1, F], F32R, tag="bc")
        nc.sync.dma_start(bcr[:], bcr_d[m].rearrange("(o g) -> o g", o=1))

        for c in range(CHUNKS_PER_MODEL):
            r0 = (m * CHUNKS_PER_MODEL + c) * CHUNK
            # --- load natural x chunk: [128, 4 tgroups, 256 feat] ---
            xn = xpool.tile([128, 4, F], F32, tag="xn")
            nc.sync.dma_start(
                xn[:], x_d[r0:r0 + CHUNK, :].rearrange("(j p) f -> p j f", p=128)
            )
            # --- PE transpose to feature-major, cast to fp32r ---
            h = []
            for fb in range(2):
                pT = psT.tile([128, CHUNK], F32, tag=f"pT_{fb}")
                for j in range(4):
                    nc.tensor.transpose(
                        pT[:, j * 128:(j + 1) * 128],
                        xn[:, j, fb * 128:(fb + 1) * 128],
                        ident[:],
                    )
                ht = hpool.tile([128, CHUNK], F32R, tag=f"h_{fb}")
                nc.vector.tensor_copy(ht[:], pT[:])
                h.append(ht)
            # --- fused layer: activations stationary -> natural-orient out ---
            on = xpool.tile([128, 4, F], F32, tag="on")
            for j in range(4):
                p4 = psL4.tile([128, F], F32, tag="p4")
                for fb in range(2):
                    nc.tensor.matmul(
                        p4[:],
                        h[fb][:, j * 128:(j + 1) * 128],
                        wc[fb][:],
                        start=(fb == 0),
                        stop=False,
                    )
                nc.tensor.matmul(p4[:], onesr[:], bcr[:], start=False, stop=True)
                # alternate copy engine to balance ACT/DVE load
                if j % 2 == 0:
                    nc.vector.tensor_copy(on[:, j, :], p4[:])
                else:
                    nc.scalar.activation(on[:, j, :], p4[:], COPY)
            nc.sync.dma_start(
                y_d[r0:r0 + CHUNK, :].rearrange("(j p) f -> p j f", p=128), on[:]
            )

    if reps == 1:
        body()
    else:
        # hardware loop: repeat the identical workload (timing harness only)
        with tc.For_i(0, reps, 1):
            body()
    ctx.close()


def build_nc(reps=1):
    nc = bacc.Bacc("TRN2", target_bir_lowering=False, debug=False,
                   num_devices=N_CORES)
    x_d = nc.dram_tensor("x", [ROWS_PER_CORE, F], F32, kind="ExternalInput").ap()
    wc_d = nc.dram_tensor("Wc", [M_PER_CORE, F, F], F32R,
                          kind="ExternalInput").ap()
    bcr_d = nc.dram_tensor("bcr", [M_PER_CORE, F], F32R,
                           kind="ExternalInput").ap()
    y_d = nc.dram_tensor("y", [ROWS_PER_CORE, F], F32, kind="ExternalOutput").ap()
    with tile.TileContext(nc) as tc:
        emit_core_kernel(tc, x_d, wc_d, bcr_d, y_d, reps=reps)
    nc.compile()
    return nc


_NC = None


def _get_nc():
    global _NC
    if _NC is None:
        _NC = build_nc()
    return _NC


def _round_f32r(a):
    """Round-to-nearest-even into the fp32r format (1+8+11 bits, top 20 of 32)."""
    u = np.ascontiguousarray(a, dtype=np.float32).view(np.uint32)
    r = (u.astype(np.uint64) + 0x7FF + ((u >> 12) & 1)) & 0xFFFFF000
    return r.astype(np.uint32).view(np.float32)


def _compose_affine(Ws, bs):
    """Fold the 4-layer affine chain into one layer per model (float64)."""
    W = np.asarray(Ws, dtype=np.float64)
    b = np.asarray(bs, dtype=np.float64)
    Wc = W[:, 0]
    bc = b[:, 0]
    for l in range(1, N_LAYERS):
        Wc = np.matmul(Wc, W[:, l])
        bc = np.matmul(bc[:, None, :], W[:, l])[:, 0] + b[:, l]
    return Wc, bc


def make_in_maps(x, Ws, bs):
    Wc, bc = _compose_affine(Ws, bs)
    Wcr = _round_f32r(Wc.astype(np.float32))
    bcr = _round_f32r(bc.astype(np.float32))
    in_maps = []
    for c in range(N_CORES):
        m0 = c * M_PER_CORE
        in_maps.append({
            "x": np.ascontiguousarray(
                x[m0 * ROWS_PER_MODEL:(m0 + M_PER_CORE) * ROWS_PER_MODEL]),
            "Wc": np.ascontiguousarray(Wcr[m0:m0 + M_PER_CORE]),
            "bcr": np.ascontiguousarray(bcr[m0:m0 + M_PER_CORE]),
        })
    return in_maps


def kernel(x, Ws, bs, slice_bounds=None, **_):
    x = np.asarray(x, dtype=np.float32)
    Ws = np.asarray(Ws, dtype=np.float32)
    bs = np.asarray(bs, dtype=np.float32)
    nc = _get_nc()
    res = run_bass_kernel_spmd(nc, make_in_maps(x, Ws, bs),
                               core_ids=list(range(N_CORES)))
    return np.concatenate([res.results[c]["y"] for c in range(N_CORES)], axis=0)


# revision 23
# speedup vs baseline: 1.9300x; 1.9300x over previous
"""Trainium2 Bass kernel for the grouped linear ensemble (moe_routing).

Problem: x [262144, 256] f32, Ws [64, 4, 256, 256], bs [64, 4, 256].
Model m applies its 4-layer stack (h = h @ W_l + b_l) to its contiguous
4096-row slice of x.

Sharding: expert parallel — core c owns models 8c..8c+7 and their rows.
No cross-device communication.

Per-core kernel design:
- Activations are kept feature-on-partitions ("transposed", hT [256, t]) so
  the contraction dim of every matmul lands on the partition axis; the
  orientation is self-consistent across chained layers (h'T = W.T @ hT).
- Input x is transposed on-chip via PE transpose-mode matmuls (exact in
  fp32), 128x128 blocks, PSUM -> SBUF copy casts to float32r.
- Layers 1-3: stationary = weight block (reused across the whole chunk),
  moving = hT, N=512 columns per matmul, fp32r (1 cycle/row at N>=512,
  ~1.5e-4 relative rounding).  Bias is fused into the PSUM->SBUF copy on
  the scalar engine (per-partition bias = per-output-feature).
- Layer 4 uses the activations as the stationary operand and streams W4,
  which makes the output land in natural row-major orientation — no
  output transpose.  Its bias is added with a K=1 ones-outer-product
  matmul accumulated into the same PSUM group.
"""

from contextlib import ExitStack

import numpy as np

import concourse.tile as tile
import concourse.mybir as mybir
from concourse import bacc
from concourse.bass_utils import run_bass_kernel_spmd
from concourse.masks import make_identity

N_CORES = 8
N_MODELS = 64
N_LAYERS = 4
F = 256
ROWS_PER_MODEL = 4096
M_PER_CORE = N_MODELS // N_CORES          # 8 models per core
ROWS_PER_CORE = M_PER_CORE * ROWS_PER_MODEL  # 32768
CHUNK = 512                               # rows of x processed per pipeline step
CHUNKS_PER_MODEL = ROWS_PER_MODEL // CHUNK   # 8

F32 = mybir.dt.float32
F32R = mybir.dt.float32r
COPY = mybir.ActivationFunctionType.Copy
IDENT = mybir.ActivationFunctionType.Identity


def emit_core_kernel(tc, x_d, wc_d, bcr_d, y_d, reps=1):
    nc = tc.nc

    ctx = ExitStack()
    const = ctx.enter_context(tc.tile_pool(name="const", bufs=1))
    wpool = ctx.enter_context(tc.tile_pool(name="w", bufs=2))
    xpool = ctx.enter_context(tc.tile_pool(name="xio", bufs=3))
    hpool = ctx.enter_context(tc.tile_pool(name="h", bufs=3))
    psT = ctx.enter_context(tc.tile_pool(name="psT", bufs=2, space="PSUM"))
    psL4 = ctx.enter_context(tc.tile_pool(name="psL4", bufs=4, space="PSUM"))

    ident = const.tile([128, 128], F32)
    make_identity(nc, ident[:])
    ones_f = const.tile([1, 128], F32)
    nc.gpsimd.memset(ones_f[:], 1.0)
    onesr = const.tile([1, 128], F32R)
    nc.vector.tensor_copy(onesr[:], ones_f[:])

    def body():
      for m in range(M_PER_CORE):
        # --- per-model composed weights (double-buffered across models) ---
        # Wc = W1@W2@W3@W4 composed in float64 on the host and rounded to
        # fp32r, so loads are pure HWDGE copies.  wc[fb] = [128 (f), 256 (g)].
        wc = []
        for fb in range(2):
            wr = wpool.tile([128, F], F32R, tag=f"wr_{fb}")
            nc.sync.dma_start(wr[:], wc_d[m, fb * 128:(fb + 1) * 128, :])
            wc.append(wr)
        # composed bias (host-rounded fp32r): [1, 256] rhs of the K=1 bias MM
        bcr = wpool.tile([1, F], F32R, tag="bc")
        nc.sync.dma_start(bcr[:], bcr_d[m].rearrange("(o g) -> o g", o=1))

        for c in range(CHUNKS_PER_MODEL):
            r0 = (m * CHUNKS_PER_MODEL + c) * CHUNK
            # --- load natural x chunk: [128, 4 tgroups, 256 feat] ---
            xn = xpool.tile([128, 4, F], F32, tag="xn")
            nc.sync.dma_start(
                xn[:], x_d[r0:r0 + CHUNK, :].rearrange("(p j) f -> p j f", j=4)
            )
            # --- PE transpose to feature-major, cast to fp32r ---
            h = []
            for fb in range(2):
                pT = psT.tile([128, CHUNK], F32, tag=f"pT_{fb}")
                for j in range(4):
                    nc.tensor.transpose(
                        pT[:, j * 128:(j + 1) * 128],
                        xn[:, j, fb * 128:(fb + 1) * 128],
                        ident[:],
                    )
                ht = hpool.tile([128, CHUNK], F32R, tag=f"h_{fb}")
                nc.vector.tensor_copy(ht[:], pT[:])
                h.append(ht)
            # --- fused layer: activations stationary -> natural-orient out ---
            on = xpool.tile([128, 4, F], F32, tag="on")
            for j in range(4):
                p4 = psL4.tile([128, F], F32, tag="p4")
                for fb in range(2):
                    nc.tensor.matmul(
                        p4[:],
                        h[fb][:, j * 128:(j + 1) * 128],
                        wc[fb][:],
                        start=(fb == 0),
                        stop=False,
                    )
                nc.tensor.matmul(p4[:], onesr[:], bcr[:], start=False, stop=True)
                # alternate copy engine to balance ACT/DVE load
                if j % 2 == 0:
                    nc.vector.tensor_copy(on[:, j, :], p4[:])
                else:
                    nc.scalar.activation(on[:, j, :], p4[:], COPY)
            nc.sync.dma_start(
                y_d[r0:r0 + CHUNK, :].rearrange("(p j) f -> p j f", j=4), on[:]
            )

    if reps == 1:
        body()
    else:
        # hardware loop: repeat the identical workload (timing harness only)
        with tc.For_i(0, reps, 1):
            body()
    ctx.close()


def build_nc(reps=1):
    nc = bacc.Bacc("TRN2", target_bir_lowering=False, debug=False,
                   num_devices=N_CORES)
    x_d = nc.dram_tensor("x", [ROWS_PER_CORE, F], F32, kind="ExternalInput").ap()
    wc_d = nc.dram_tensor("Wc", [M_PER_CORE, F, F], F32R,
                          kind="ExternalInput").ap()
    bcr_d = nc.dram_tensor("bcr", [M_PER_CORE, F], F32R,
                           kind="ExternalInput").ap()
    y_d = nc.dram_tensor("y", [ROWS_PER_CORE, F], F32, kind="ExternalOutput").ap()
    with tile.TileContext(nc) as tc:
        emit_core_kernel(tc, x_d, wc_d, bcr_d, y_d, reps=reps)
    nc.compile()
    return nc


_NC = None


def _get_nc():
    global _NC
    if _NC is None:
        _NC = build_nc()
    return _NC


def _round_f32r(a):
    """Round-to-nearest-even into the fp32r format (1+8+11 bits, top 20 of 32)."""
    u = np.ascontiguousarray(a, dtype=np.float32).view(np.uint32)
    r = (u.astype(np.uint64) + 0x7FF + ((u >> 12) & 1)) & 0xFFFFF000
    return r.astype(np.uint32).view(np.float32)


def _compose_affine(Ws, bs):
    """Fold the 4-layer affine chain into one layer per model (float64)."""
    W = np.asarray(Ws, dtype=np.float64)
    b = np.asarray(bs, dtype=np.float64)
    Wc = W[:, 0]
    bc = b[:, 0]
    for l in range(1, N_LAYERS):
        Wc = np.matmul(Wc, W[:, l])
        bc = np.matmul(bc[:, None, :], W[:, l])[:, 0] + b[:, l]
    return Wc, bc


def make_in_maps(x, Ws, bs):
    Wc, bc = _compose_affine(Ws, bs)
    Wcr = _round_f32r(Wc.astype(np.float32))
    bcr = _round_f32r(bc.astype(np.float32))
    in_maps = []
    for c in range(N_CORES):
        m0 = c * M_PER_CORE
        in_maps.append({
            "x": np.ascontiguousarray(
                x[m0 * ROWS_PER_MODEL:(m0 + M_PER_CORE) * ROWS_PER_MODEL]),
            "Wc": np.ascontiguousarray(Wcr[m0:m0 + M_PER_CORE]),
            "bcr": np.ascontiguousarray(bcr[m0:m0 + M_PER_CORE]),
        })
    return in_maps


def kernel(x, Ws, bs, slice_bounds=None, **_):
    x = np.asarray(x, dtype=np.float32)
    Ws = np.asarray(Ws, dtype=np.float32)
    bs = np.asarray(bs, dtype=np.float32)
    nc = _get_nc()
    res = run_bass_kernel_spmd(nc, make_in_maps(x, Ws, bs),
                               core_ids=list(range(N_CORES)))
    return np.concatenate([res.results[c]["y"] for c in range(N_CORES)], axis=0)
